# revision 7
# baseline (speedup 1.0000x reference)
"""Trainium2 Bass kernel for nn_BaselineModel_75256416960594 (retrieval_knn).

Computes, for feat_map (1,128,64,64) and feature_bank (50000,128):
    flat = l2_normalize(feat_map reshaped to (4096,128))
    d2[p,m] = ||flat_p||^2 + ||bank_m||^2 - 2 flat_p . bank_m
    patch_scores = sqrt(max(min_m d2, 0)) reshaped (64,64)
    anomaly_map = bilinear_upsample(patch_scores, 512, 512)  (half-pixel)
    anomaly_score = max(anomaly_map)

Sharding: feature_bank rows split across 8 NeuronCores (6250 rows each,
padded to 6272); patches replicated. Each core computes a partial min
over its shard; an AllReduce(min) combines partials; every core then
finishes the (tiny) sqrt/upsample/max tail identically.

Per-core dataflow (bank rows on partitions, patches on free dim):
  G_tile[128,512] (PSUM, f32) = bankT_chunk.T @ (-2*rn*flatT)   [fp16 matmul]
  ~73% of tiles: ACT Identity(bias=b2_chunk) drains PSUM->fp16 SBUF, then
                 DVE tensor_tensor(min) at 2x fp16 mode updates running min.
  ~27% of tiles: b2 is pre-accumulated into PSUM via a K=1 ones-matmul and
                 DVE takes the min directly from PSUM (f32).
  Partition-axis min at the end via PE transposes + DVE reduce_min.
"""

import functools

import numpy as np

import concourse.bacc as bacc
import concourse.mybir as mybir
import concourse.tile as tile
from concourse.bass_utils import run_bass_kernel_spmd
from concourse.masks import make_identity

N_CORES = 8
C = 128            # feature channels
NPATCH = 4096      # 64*64 patches
HW = 64
OUT = 512
BANK = 50000
SHARD = BANK // N_CORES          # 6250
NQ = 49                          # bank chunks per core (49*128 = 6272)
PAD_SHARD = NQ * 128             # 6272
NT = NPATCH // 512               # 8 patch chunks of 512
PAD_VAL = 15.5                   # pad rows: b2 = 128*15.5^2 = 30752 >> any real V
RINIT = 6.0e4                    # running-min init (fp16-representable, > pad V)

F16 = mybir.dt.float16
F32 = mybir.dt.float32

# every DIRECT_EVERYth tile (in flattened (q,t) order) takes the DVE-direct
# path (b2 via extra PE matmul + min straight from PSUM); the rest drain
# through ACT. 4 -> 25% direct, balancing ACT vs DVE occupancy.
DIRECT_EVERY = 4


def _resize_matrix(out_size: int, in_size: int) -> np.ndarray:
    """Row-normalized triangle-kernel weights == jax.image.resize bilinear
    (half-pixel centers, upsampling)."""
    scale = in_size / out_size
    x = (np.arange(out_size) + 0.5) * scale - 0.5
    w = np.maximum(0.0, 1.0 - np.abs(x[:, None] - np.arange(in_size)[None, :]))
    w = w / w.sum(axis=1, keepdims=True)
    return w.astype(np.float32)


def _build(debug=False):
    nc = bacc.Bacc(num_devices=N_CORES)

    flatT = nc.dram_tensor("flatT", [C, NPATCH], F32, kind="ExternalInput")
    bankT = nc.dram_tensor("bankT", [C, PAD_SHARD], F16, kind="ExternalInput")
    LT = nc.dram_tensor("LT", [HW, OUT], F32, kind="ExternalInput")
    out_map = nc.dram_tensor("out_map", [OUT, OUT], F32, kind="ExternalOutput")
    out_score = nc.dram_tensor("out_score", [1, 1], F32, kind="ExternalOutput")

    a2_dram = nc.dram_tensor("a2_dram", [1, NPATCH], F32, kind="Internal")
    b2_dram = nc.dram_tensor("b2_dram", [1, PAD_SHARD], F32, kind="Internal")
    md_dram = nc.dram_tensor("md_dram", [1, NPATCH], F32, kind="Internal")
    md_red = nc.dram_tensor("md_red", [1, NPATCH], F32, kind="Internal",
                            addr_space="Shared")
    sc_dram = nc.dram_tensor("sc_dram", [128, 1], F32, kind="Internal")
    if debug:
        dbg_md = nc.dram_tensor("dbg_md", [128, 32], F32, kind="ExternalOutput")
        dbg_b2 = nc.dram_tensor("dbg_b2", [1, PAD_SHARD], F32, kind="ExternalOutput")
        dbg_rn = nc.dram_tensor("dbg_rn", [1, NPATCH], F32, kind="ExternalOutput")
        dbg_a2 = nc.dram_tensor("dbg_a2", [1, NPATCH], F32, kind="ExternalOutput")
        dbg_R = nc.dram_tensor("dbg_R", [128, NPATCH], F32, kind="ExternalOutput")
        dbg_f2 = nc.dram_tensor("dbg_f2", [C, 512], F32, kind="ExternalOutput")
        dbg_S2 = nc.dram_tensor("dbg_S2", [HW, HW], F32, kind="ExternalOutput")
        dbg_mdred = nc.dram_tensor("dbg_mdred", [128, 32], F32, kind="ExternalOutput")

    with tile.TileContext(nc) as tc:
        with tc.tile_pool(name="persist", bufs=1) as pp, \
             tc.tile_pool(name="vbuf", bufs=6) as vp, \
             tc.tile_pool(name="ps_main", bufs=6, space="PSUM") as psm, \
             tc.tile_pool(name="ps_small", bufs=2, space="PSUM") as pss:

            # ---------------- load inputs (split DMAs for queue parallelism)
            flatT_sb = pp.tile([C, NPATCH], F32)
            bankT_sb = pp.tile([C, PAD_SHARD], F16)
            LT_sb = pp.tile([HW, OUT], F32)
            for j in range(8):
                nc.sync.dma_start(out=flatT_sb[:, j * 512:(j + 1) * 512],
                                  in_=flatT[:, j * 512:(j + 1) * 512])
            for j in range(8):
                s0, s1 = j * 784, min(PAD_SHARD, (j + 1) * 784)
                nc.sync.dma_start(out=bankT_sb[:, s0:s1], in_=bankT[:, s0:s1])
            nc.sync.dma_start(out=LT_sb, in_=LT[:, :])

            ones_col = pp.tile([C, 1], F32)       # f32 ones column (nsq matmul)
            nc.vector.memset(ones_col, 1.0)
            ones_col16 = pp.tile([C, 1], F16)     # fp16 ones column (b2 matmul)
            nc.vector.memset(ones_col16, 1.0)
            neg2_col = pp.tile([1, C], F32)       # -2 row (rn broadcast matmul)
            nc.vector.memset(neg2_col, -2.0)
            ones_row16 = pp.tile([1, 512], F16)   # fp16 ones row (b2 bcast matmul)
            nc.vector.memset(ones_row16, 1.0)

            # ---------------- normalization: rn = 1/max(||x_p||, eps)
            sq_sb = pp.tile([C, NPATCH], F32)
            nc.vector.tensor_tensor(out=sq_sb, in0=flatT_sb, in1=flatT_sb,
                                    op=mybir.AluOpType.mult)
            nsq_sb = pp.tile([1, NPATCH], F32)
            for j in range(NT):
                ps = pss.tile([1, 512], F32, tag="s")
                nc.tensor.matmul(ps, ones_col[:, :], sq_sb[:, j * 512:(j + 1) * 512],
                                 start=True, stop=True)
                nc.scalar.copy(nsq_sb[:, j * 512:(j + 1) * 512], ps)
            s_sb = pp.tile([1, NPATCH], F32)
            nc.scalar.activation(s_sb, nsq_sb, mybir.ActivationFunctionType.Sqrt)
            nc.vector.tensor_scalar_max(s_sb, s_sb, 1.0e-12)
            rn_sb = pp.tile([1, NPATCH], F32)
            nc.vector.reciprocal(rn_sb, s_sb)

            # a2 = nsq * rn^2  (== ||flat_p||^2 after normalize, ~1.0)
            a2_sb = pp.tile([1, NPATCH], F32)
            nc.vector.tensor_tensor(out=a2_sb, in0=nsq_sb, in1=rn_sb,
                                    op=mybir.AluOpType.mult)
            nc.vector.tensor_tensor(out=a2_sb, in0=a2_sb, in1=rn_sb,
                                    op=mybir.AluOpType.mult)
            nc.sync.dma_start(out=a2_dram[:, :], in_=a2_sb)

            # flatT2 = (-2 * rn) broadcast * flatT   -> fp16
            flatT2 = pp.tile([C, NPATCH], F16)
            for j in range(NT):
                rb = pss.tile([128, 512], F32, tag="s")
                nc.tensor.matmul(rb, neg2_col[:, :], rn_sb[:, j * 512:(j + 1) * 512],
                                 start=True, stop=True)
                nc.vector.tensor_tensor(out=flatT2[:, j * 512:(j + 1) * 512],
                                        in0=flatT_sb[:, j * 512:(j + 1) * 512],
                                        in1=rb, op=mybir.AluOpType.mult)

            # ---------------- b2 = rowsum(bank^2), from fp16 bankT
            sqb_sb = pp.tile([C, PAD_SHARD], F16)
            nc.vector.tensor_tensor(out=sqb_sb, in0=bankT_sb, in1=bankT_sb,
                                    op=mybir.AluOpType.mult)
            b2f_sb = pp.tile([1, PAD_SHARD], F32)   # f32 (per-partition bias use)
            b2h_sb = pp.tile([1, PAD_SHARD], F16)   # fp16 (b2-matmul lhsT use)
            for j in range((PAD_SHARD + 511) // 512):
                s0, s1 = j * 512, min(PAD_SHARD, (j + 1) * 512)
                ps = pss.tile([1, 512], F32, tag="s")
                nc.tensor.matmul(ps[:, :s1 - s0], ones_col16[:, :], sqb_sb[:, s0:s1],
                                 start=True, stop=True)
                nc.scalar.copy(b2f_sb[:, s0:s1], ps[:, :s1 - s0])
                nc.scalar.copy(b2h_sb[:, s0:s1], ps[:, :s1 - s0])
            # per-partition layout b2pp[q, j] = b2[j*128+q] via DRAM roundtrip
            nc.sync.dma_start(out=b2_dram[:, :], in_=b2f_sb)
            b2pp = pp.tile([128, NQ], F32)
            nc.sync.dma_start(
                out=b2pp,
                in_=b2_dram[:, :].rearrange("a (j q) -> (a q) j", q=128))

            # ---------------- main loop: running min over bank chunks
            R = pp.tile([128, NPATCH], F16)
            nc.vector.memset(R, RINIT)
            k = 0
            for q in range(NQ):
                for t in range(NT):
                    G = psm.tile([128, 512], F32, tag="G")
                    direct = (k % DIRECT_EVERY == 0)
                    k += 1
                    if direct:
                        nc.tensor.matmul(G, b2h_sb[:, q * 128:(q + 1) * 128],
                                         ones_row16[:, :], start=True, stop=False)
                        nc.tensor.matmul(G, bankT_sb[:, q * 128:(q + 1) * 128],
                                         flatT2[:, t * 512:(t + 1) * 512],
                                         start=False, stop=True)
                        nc.vector.tensor_tensor(
                            out=R[:, t * 512:(t + 1) * 512],
                            in0=R[:, t * 512:(t + 1) * 512], in1=G,
                            op=mybir.AluOpType.min)
                    else:
                        nc.tensor.matmul(G, bankT_sb[:, q * 128:(q + 1) * 128],
                                         flatT2[:, t * 512:(t + 1) * 512],
                                         start=True, stop=True)
                        V = vp.tile([128, 512], F16, tag="V")
                        nc.scalar.activation(V, G,
                                             mybir.ActivationFunctionType.Identity,
                                             bias=b2pp[:, q:q + 1], scale=1.0)
                        nc.vector.tensor_tensor(
                            out=R[:, t * 512:(t + 1) * 512],
                            in0=R[:, t * 512:(t + 1) * 512], in1=V,
                            op=mybir.AluOpType.min)

            # ---------------- partition-axis min: PE transpose + reduce_min
            ident16 = pp.tile([128, 128], F16)
            make_identity(nc, ident16)
            md_sb = pp.tile([128, 32], F32)
            for tc_i in range(32):
                tp = pss.tile([128, 128], F16, tag="s")
                nc.tensor.transpose(tp, R[:, tc_i * 128:(tc_i + 1) * 128],
                                    ident16[:, :])
                nc.vector.tensor_reduce(out=md_sb[:, tc_i:tc_i + 1], in_=tp,
                                        axis=mybir.AxisListType.X,
                                        op=mybir.AluOpType.min)
            nc.sync.dma_start(
                out=md_dram[:, :].rearrange("o (t p) -> (o p) t", p=128),
                in_=md_sb)
            if debug:
                nc.sync.dma_start(out=dbg_md[:, :], in_=md_sb)
                nc.sync.dma_start(out=dbg_b2[:, :], in_=b2f_sb)
                nc.sync.dma_start(out=dbg_rn[:, :], in_=rn_sb)
                nc.sync.dma_start(out=dbg_a2[:, :], in_=a2_sb)
                for jj in range(NT):
                    Rf32 = vp.tile([128, 512], F32, tag="dbgR")
                    nc.scalar.copy(Rf32, R[:, jj * 512:(jj + 1) * 512])
                    nc.sync.dma_start(out=dbg_R[:, jj * 512:(jj + 1) * 512],
                                      in_=Rf32)
                f2f = vp.tile([C, 512], F32, tag="dbgR")
                nc.scalar.copy(f2f, flatT2[:, 0:512])
                nc.sync.dma_start(out=dbg_f2[:, :], in_=f2f)

            # ---------------- AllReduce(min) across the 8 cores
            nc.gpsimd.collective_compute(
                "AllReduce", mybir.AluOpType.min,
                replica_groups=[list(range(N_CORES))],
                ins=[md_dram[:, :].opt()], outs=[md_red[:, :].opt()])

            if debug:
                mr_sb = pp.tile([128, 32], F32)
                nc.sync.dma_start(
                    out=mr_sb,
                    in_=md_red[:, :].rearrange("o (t p) -> (o p) t", p=128))
                nc.sync.dma_start(out=dbg_mdred[:, :], in_=mr_sb)
            # ---------------- tail: d = sqrt(max(a2 + min_d2, 0)), upsample, max
            # md_red[p, a]: global patch idx = a*128 + p = h*64 + w,
            # h = 2a + b, p = 64b + w  ->  S2[h, w]
            S2 = pp.tile([HW, HW], F32)
            nc.sync.dma_start(
                out=S2,
                in_=md_red[:, :].rearrange("o (h w) -> (o h) w", w=HW))
            a2_64 = pp.tile([HW, HW], F32)
            nc.sync.dma_start(out=a2_64,
                              in_=a2_dram[:, :].rearrange("o (h w) -> (o h) w", w=HW))
            nc.vector.tensor_tensor(out=S2, in0=S2, in1=a2_64,
                                    op=mybir.AluOpType.add)
            if debug:
                nc.sync.dma_start(out=dbg_S2[:, :], in_=S2)
            nc.vector.tensor_scalar_max(S2, S2, 0.0)
            # sqrt + one Newton step (ACT sqrt table has a loose error budget)
            S = pp.tile([HW, HW], F32)
            nc.scalar.activation(S, S2, mybir.ActivationFunctionType.Sqrt)
            Sc = pp.tile([HW, HW], F32)
            nc.vector.tensor_scalar_max(Sc, S, 1.0e-6)
            Rc = pp.tile([HW, HW], F32)
            nc.vector.reciprocal(Rc, Sc)
            nc.vector.tensor_tensor(out=Rc, in0=S2, in1=Rc,
                                    op=mybir.AluOpType.mult)   # S2/S
            nc.vector.tensor_tensor(out=S, in0=S, in1=Rc,
                                    op=mybir.AluOpType.add)     # S + S2/S
            nc.vector.tensor_scalar_mul(S, S, 0.5)              # 0.5*(S + S2/S)

            # transpose S for the first upsample matmul
            ident32 = pp.tile([HW, HW], F32)
            make_identity(nc, ident32)
            St_ps = pss.tile([HW, HW], F32, tag="s")
            nc.tensor.transpose(St_ps, S[:, :], ident32[:, :])
            St = pp.tile([HW, HW], F32)
            nc.scalar.copy(St, St_ps)
            # A = S @ L^T  [64, 512]
            A_ps = pss.tile([HW, OUT], F32, tag="s")
            nc.tensor.matmul(A_ps, St[:, :], LT_sb[:, :], start=True, stop=True)
            A_sb = pp.tile([HW, OUT], F32)
            nc.scalar.copy(A_sb, A_ps)
            # U = L @ A  [512, 512], in 4 chunks of 128 rows
            U_sb = pp.tile([128, 4 * OUT], F32)
            for c4 in range(4):
                U_ps = psm.tile([128, OUT], F32, tag="G")
                nc.tensor.matmul(U_ps, LT_sb[:, c4 * 128:(c4 + 1) * 128],
                                 A_sb[:, :], start=True, stop=True)
                nc.scalar.copy(U_sb[:, c4 * OUT:(c4 + 1) * OUT], U_ps)
            nc.sync.dma_start(
                out=out_map[:, :].rearrange("(c p) j -> p c j", p=128),
                in_=U_sb[:, :].rearrange("p (c j) -> p c j", j=OUT))

            # anomaly_score = max over the upsampled map
            rmax = pp.tile([128, 1], F32)
            nc.vector.tensor_reduce(out=rmax, in_=U_sb,
                                    axis=mybir.AxisListType.X,
                                    op=mybir.AluOpType.max)
            nc.sync.dma_start(out=sc_dram[:, :], in_=rmax)
            rrow = pp.tile([1, 128], F32)
            nc.sync.dma_start(out=rrow,
                              in_=sc_dram[:, :].rearrange("p a -> a p"))
            s11 = pp.tile([1, 1], F32)
            nc.vector.tensor_reduce(out=s11, in_=rrow,
                                    axis=mybir.AxisListType.X,
                                    op=mybir.AluOpType.max)
            nc.sync.dma_start(out=out_score[:, :], in_=s11)

    nc.finalize()
    return nc


@functools.lru_cache(maxsize=2)
def _get_nc(debug=False):
    return _build(debug)


def kernel(feat_map, feature_bank, out_size):
    assert int(out_size) == OUT, f"kernel hardcodes out_size={OUT}"
    feat_map = np.asarray(feat_map, dtype=np.float32)
    feature_bank = np.asarray(feature_bank, dtype=np.float32)
    assert feat_map.shape == (1, C, HW, HW)
    assert feature_bank.shape == (BANK, C)

    flatT_np = np.ascontiguousarray(feat_map.reshape(C, NPATCH))
    LT_np = np.ascontiguousarray(_resize_matrix(OUT, HW).T)  # [64, 512]

    in_maps = []
    for c in range(N_CORES):
        shard = feature_bank[c * SHARD:(c + 1) * SHARD]          # [6250, 128]
        pad = np.full((PAD_SHARD - SHARD, C), PAD_VAL, np.float32)
        shard = np.concatenate([shard, pad], axis=0)             # [6272, 128]
        bankT_np = np.ascontiguousarray(shard.T.astype(np.float16))
        in_maps.append({"flatT": flatT_np, "bankT": bankT_np, "LT": LT_np})

    nc = _get_nc()
    res = run_bass_kernel_spmd(nc, in_maps, core_ids=list(range(N_CORES)))
    r0 = res.results[0]
    anomaly_map = np.asarray(r0["out_map"], dtype=np.float32)
    anomaly_score = np.float32(np.asarray(r0["out_score"]).reshape(()))
    return anomaly_map, anomaly_score
